# revision 10
# baseline (speedup 1.0000x reference)
"""Trainium2 Bass kernel for nn_Classifier_5712306504361 (LorentzGIN classifier).

Distribution (8 NeuronCores, dst-sharded graph parallel):
  - Host: log-map scale s folded into a bf16 tangent table xt = [0, s*tail];
    edges sorted by dst, grouped into 128-edge tiles per 128-dst block;
    tiny weights replicated with bias folded into row 0 (the always-zero
    feature-0 lane carries a constant 1).
  - Device, per core: indirect-DMA gathers of xt rows per edge tile; one-hot
    sel matrices scatter-add the segment sum on the PE (bf16); the GIN MLP
    runs node-major with DMA-XBAR transposes feeding lhsT and W as the
    moving operand, so matmul output is node-major with bias pre-added.
    All exp/log-map pairs collapse algebraically to u = min(50/|v|, 1)
    (log_map_zero(exp_map_zero(v)) == v * min(|v|,50)/|v|, and the
    post-relu renormalization is exactly 1), computed via
    activation(Square, accum_out=t2) + sqrt + reciprocal.
  - Host: sum the 8 partial [384] pool vectors, mean, tiny classify+softmax
    epilogue on a [10]-vector.
"""
import sys
import numpy as np

sys.path.insert(0, "/opt/trn_rl_repo")

P = 128
EPS = 1e-7

DEFAULT_CFG = dict(
    NCORES=8,
    NLOC=6250,     # real nodes per core
    NBLK=49,       # 128-dst blocks per core (NLOC <= NBLK*128)
    CHUNK=8,       # blocks per super-chunk
    TK=16,         # edge tiles per indirect gather call
    HB=3,          # blocks per PSUM sub-batch in MLP layers
    DMAT=False,    # use DMA XBAR transpose (else PE transpose)
    SELSPLIT=False,  # alternate sel one-hot builds between DVE and GpSimd (GpSimd is ~12x slower - keep off)
)


def _derive(cfg):
    d = dict(cfg)
    d["N"] = d["NCORES"] * d["NLOC"]
    d["NLOC_PAD"] = d["NBLK"] * P
    d["NTAB"] = ((d["N"] + 1 + P - 1) // P) * P
    d["ZROW"] = d["N"]
    d["MASK_LIM"] = d["NLOC"] - (d["NBLK"] - 1) * P  # real nodes in last block
    return d


# ---------------------------------------------------------------------------
# host-side preprocessing (data formatting only)
# ---------------------------------------------------------------------------

def host_prep(x, edge_index, cfg, bf_np):
    c = _derive(cfg)
    N, NTAB, NLOC = c["N"], c["NTAB"], c["NLOC"]
    NBLK, ZROW, NLOC_PAD, NCORES = c["NBLK"], c["ZROW"], c["NLOC_PAD"], c["NCORES"]

    x = np.ascontiguousarray(np.asarray(x, np.float32))
    ei = np.asarray(edge_index).astype(np.int64)
    src, dst = ei[0], ei[1]

    # log_map_zero of every node, mirroring the fp32 reference math:
    # s = arcosh(max(y0+EPS, 1+EPS)) / sqrt(sum(tail^2)+EPS); xt = [0, s*tail]
    y0 = x[:, 0]
    z = np.maximum(y0 + np.float32(EPS), np.float32(1.0 + EPS)).astype(np.float32)
    dist = np.log(z + np.sqrt(z * z - np.float32(1.0))).astype(np.float32)
    t2 = np.square(x[:, 1:]).sum(axis=1, dtype=np.float32)
    s = dist / np.sqrt(t2 + np.float32(EPS))
    xt = np.zeros((NTAB, P), np.float32)
    xt[:N, 1:] = x[:, 1:] * s[:, None]
    xt_bf = np.ascontiguousarray(xt.astype(bf_np))

    order = np.argsort(dst, kind="stable")
    src_s, dst_s = src[order], dst[order]

    per_core = []
    Kb = np.ones(NBLK, np.int64)
    for ci in range(NCORES):
        lo = ci * NLOC
        bounds = [np.searchsorted(dst_s, lo + min(b * P, NLOC)) for b in range(NBLK + 1)]
        segs = []
        for b in range(NBLK):
            s0, s1 = int(bounds[b]), int(bounds[b + 1])
            segs.append((s0, s1))
            Kb[b] = max(Kb[b], (s1 - s0 + P - 1) // P)
        per_core.append((lo, segs))

    T = int(Kb.sum())
    cores = []
    for ci in range(NCORES):
        lo, segs = per_core[ci]
        idx = np.full((P, T), ZROW, np.int32)
        slot = np.zeros((P, T), np.float32)
        col = 0
        for b in range(NBLK):
            s0, s1 = segs[b]
            k = s1 - s0
            kb = int(Kb[b])
            ps = np.full(kb * P, ZROW, np.int64)
            ps[:k] = src_s[s0:s1]
            sl = np.zeros(kb * P, np.float32)
            sl[:k] = (dst_s[s0:s1] - lo - b * P).astype(np.float32)
            idx[:, col:col + kb] = ps.reshape(kb, P).T
            slot[:, col:col + kb] = sl.reshape(kb, P).T
            col += kb
        own_ids = np.arange(lo, lo + NLOC_PAD)
        own_ids = np.where(own_ids < N, own_ids, ZROW)
        own = np.ascontiguousarray(xt[own_ids, :])            # [NLOC_PAD, 128] fp32
        cores.append(dict(idx=idx, slot=np.ascontiguousarray(slot.astype(bf_np)), own=own))
    return xt_bf, [int(v) for v in Kb], cores


def prep_weights(W0, b0, W1, b1, W2, b2, bf_np):
    """W as [k_in, m_out] with zero col 0 and bias in row 0 (ones-lane)."""
    def wr(W, b, ki, mo):
        w = np.zeros((ki, mo), np.float32)
        W = np.asarray(W, np.float32)
        b = np.asarray(b, np.float32)
        w[1:W.shape[1] + 1, 1:W.shape[0] + 1] = W.T
        w[0, 1:len(b) + 1] = b
        return w

    w2 = wr(W2, b2, 256, 384)
    return dict(w0=wr(W0, b0, P, P).astype(bf_np),
                w1=wr(W1, b1, P, 256).astype(bf_np),
                w2a=np.ascontiguousarray(w2[:P]).astype(bf_np),
                w2b=np.ascontiguousarray(w2[P:]).astype(bf_np))


# ---------------------------------------------------------------------------
# device program
# ---------------------------------------------------------------------------

def build_program(Kb, cfg):
    import concourse.bass as bass
    import concourse.tile as tile
    from concourse import mybir
    from concourse.masks import make_identity
    from contextlib import ExitStack

    c = _derive(cfg)
    NTAB, NBLK, CHUNK, TK, HB = c["NTAB"], c["NBLK"], c["CHUNK"], c["TK"], c["HB"]
    MASK_LIM, NLOC_PAD = c["MASK_LIM"], c["NLOC_PAD"]
    F32 = mybir.dt.float32
    I32 = mybir.dt.int32
    BF = mybir.dt.bfloat16
    F8 = mybir.dt.float8e4
    AF = mybir.ActivationFunctionType
    OP = mybir.AluOpType
    T = int(sum(Kb))

    nc = bass.Bass("TRN2", debug=False, num_devices=c["NCORES"])

    xt_d = nc.dram_tensor("xt", [NTAB, P], BF, kind="ExternalInput")
    idx_d = nc.dram_tensor("idx", [P, T], I32, kind="ExternalInput")
    slot_d = nc.dram_tensor("slot", [P, T], BF, kind="ExternalInput")
    own_d = nc.dram_tensor("own", [NLOC_PAD, P], F32, kind="ExternalInput")
    w0_d = nc.dram_tensor("w0", [P, P], BF, kind="ExternalInput")
    w1_d = nc.dram_tensor("w1", [P, 256], BF, kind="ExternalInput")
    w2a_d = nc.dram_tensor("w2a", [P, 384], BF, kind="ExternalInput")
    w2b_d = nc.dram_tensor("w2b", [P, 384], BF, kind="ExternalInput")
    out_d = nc.dram_tensor("out", [P, 3], F32, kind="ExternalOutput")

    chunks = []
    b0 = 0
    while b0 < NBLK:
        nb = min(CHUNK, NBLK - b0)
        chunks.append((b0, nb))
        b0 += nb
    tile_col = np.concatenate([[0], np.cumsum(Kb)]).astype(int)

    LAYERS = [(1, 1), (1, 2), (2, 3)]   # (ktiles, jtiles) per layer

    with tile.TileContext(nc) as tc, ExitStack() as ctx:
        consts = ctx.enter_context(tc.tile_pool(name="consts", bufs=1))
        edgep = ctx.enter_context(tc.tile_pool(name="edgep", bufs=3))
        gath = ctx.enter_context(tc.tile_pool(name="gath", bufs=4))
        ownp = ctx.enter_context(tc.tile_pool(name="ownp", bufs=3))
        wide = ctx.enter_context(tc.tile_pool(name="wide", bufs=3))
        sc = ctx.enter_context(tc.tile_pool(name="sc", bufs=4))
        scr = ctx.enter_context(tc.tile_pool(name="scr", bufs=4))
        psA = ctx.enter_context(tc.tile_pool(name="psA", bufs=2, space="PSUM"))
        psM = ctx.enter_context(tc.tile_pool(name="psM", bufs=3, space="PSUM"))
        psT = ctx.enter_context(tc.tile_pool(name="psT", bufs=2, space="PSUM"))
        psP = ctx.enter_context(tc.tile_pool(name="psP", bufs=1, space="PSUM"))

        # ---- constants ----
        KMAX = int(max(Kb))
        iota_i = consts.tile([P, KMAX * P], I32)
        nc.gpsimd.iota(iota_i[:], pattern=[[0, KMAX], [1, P]], base=0,
                       channel_multiplier=0)
        iota_bf = consts.tile([P, KMAX * P], BF)
        nc.vector.tensor_copy(out=iota_bf[:], in_=iota_i[:])
        ident_bf = consts.tile([P, P], BF)
        make_identity(nc, ident_bf[:])
        w0_sb = consts.tile([P, P], BF)
        nc.sync.dma_start(out=w0_sb[:], in_=w0_d[:])
        w1_sb = consts.tile([P, 256], BF)
        nc.sync.dma_start(out=w1_sb[:], in_=w1_d[:])
        w2a_sb = consts.tile([P, 384], BF)
        nc.sync.dma_start(out=w2a_sb[:], in_=w2a_d[:])
        w2b_sb = consts.tile([P, 384], BF)
        nc.sync.dma_start(out=w2b_sb[:], in_=w2b_d[:])
        ones_bf = consts.tile([P, 1], BF)
        nc.vector.memset(ones_bf[:], 1.0)
        eps_col = consts.tile([P, 1], F32)
        nc.vector.memset(eps_col[:], EPS)
        mask_i = consts.tile([P, 1], I32)
        nc.gpsimd.iota(mask_i[:], pattern=[[0, 1]], base=0, channel_multiplier=1)
        mask_bf = consts.tile([P, 1], BF)
        nc.vector.tensor_scalar(out=mask_bf[:], in0=mask_i[:], scalar1=MASK_LIM,
                                scalar2=None, op0=OP.is_lt)

        def bcast(ap2d, f):
            """[P, w] AP -> broadcast AP [P, w, f] (0-step inner dim)."""
            return bass.AP(tensor=ap2d.tensor, offset=ap2d.offset,
                           ap=[ap2d.ap[0], ap2d.ap[1], [0, f]])

        def scale_chain(t2ap, w, tag):
            """u = min(|v|,50)/|v| with |v| = max(sqrt(t2+EPS), 1e-3)."""
            nr = sc.tile([P, w], F32, tag=tag + "n")
            nc.scalar.activation(nr[:], t2ap, AF.Sqrt, bias=eps_col[:, 0:1])
            n = sc.tile([P, w], F32, tag=tag + "m")
            nc.vector.tensor_scalar(out=n[:], in0=nr[:], scalar1=1e-3,
                                    scalar2=None, op0=OP.max)
            rcp = sc.tile([P, w], F32, tag=tag + "r")
            nc.vector.reciprocal(rcp[:], n[:])
            u = sc.tile([P, w], F32, tag=tag + "u")
            nc.vector.tensor_scalar(out=u[:], in0=rcp[:], scalar1=50.0,
                                    scalar2=1.0, op0=OP.mult, op1=OP.min)
            return u

        Wl = [[w0_sb], [w1_sb], [w2a_sb, w2b_sb]]
        pool_ps = psP.tile([P, 4], F32)
        GR = int(c.get("GR", 4))

        def emit_gin(cb0, nb):
            t0, t1 = int(tile_col[cb0]), int(tile_col[cb0 + nb])
            ntc = t1 - t0

            idx_sb = edgep.tile([P, ntc], I32, tag="idx")
            nc.sync.dma_start(out=idx_sb[:], in_=idx_d[:, t0:t1])
            slot_sb = edgep.tile([P, ntc], BF, tag="slot")
            nc.sync.dma_start(out=slot_sb[:], in_=slot_d[:, t0:t1])

            gtiles = []
            for g0 in range(0, ntc, TK):
                gk = min(TK, ntc - g0)
                gt = gath.tile([P, TK * P], BF, tag="gath")
                nc.gpsimd.indirect_dma_start(
                    out=gt[:, :gk * P],
                    out_offset=None,
                    in_=xt_d[:, :],
                    in_offset=bass.IndirectOffsetOnAxis(ap=idx_sb[:, g0:g0 + gk], axis=0),
                )
                gtiles.append(gt)

            own_sb = ownp.tile([P, nb * P], F32, tag="own")
            nc.sync.dma_start(
                out=own_sb[:].rearrange("p (t f) -> p t f", t=nb),
                in_=own_d[cb0 * P:(cb0 + nb) * P, :].rearrange("(t p) f -> p t f", p=P))

            # scatter-add via one-hot PE matmuls; sel one-hots built GR tiles
            # per DVE instruction
            out0 = wide.tile([P, nb * P], F32, tag="out0")
            t2g = sc.tile([P, CHUNK], F32, tag="t2g")
            for bi in range(nb):
                b = cb0 + bi
                ntb = int(tile_col[b + 1] - tile_col[b])
                agg_ps = psA.tile([P, P], F32, tag="agg")
                tl0 = int(tile_col[b]) - t0
                for ti0 in range(0, ntb, GR):
                    gn = min(GR, ntb - ti0)
                    selw = edgep.tile([P, GR * P], F8, tag="selw")
                    nc.vector.tensor_tensor(
                        out=selw[:, :gn * P].rearrange("p (t f) -> p t f", t=gn),
                        in0=iota_bf[:, :gn * P].rearrange("p (t f) -> p t f", t=gn),
                        in1=bcast(slot_sb[:, tl0 + ti0:tl0 + ti0 + gn], P),
                        op=OP.is_equal)
                    for ti in range(ti0, ti0 + gn):
                        tloc = tl0 + ti
                        gt = gtiles[tloc // TK]
                        off = (tloc % TK) * P
                        nc.tensor.matmul(
                            out=agg_ps[:],
                            lhsT=selw[:, (ti - ti0) * P:(ti - ti0 + 1) * P],
                            rhs=gt[:, off:off + P],
                            start=(ti == 0), stop=(ti == ntb - 1))
                nc.vector.tensor_tensor(out=out0[:, bi * P:(bi + 1) * P],
                                        in0=agg_ps[:],
                                        in1=own_sb[:, bi * P:(bi + 1) * P],
                                        op=OP.add)
                sqg = scr.tile([P, P], BF, tag="sqg")
                nc.scalar.activation(sqg[:], out0[:, bi * P:(bi + 1) * P],
                                     AF.Square, accum_out=t2g[:, bi:bi + 1])
            return out0, t2g

        def emit_mlp(cb0, nb, out0, t2g):
            u0 = scale_chain(t2g[:, :nb], nb, "cg")
            xin = wide.tile([P, nb * P], BF, tag="xin0")
            for bi in range(nb):
                nc.vector.tensor_scalar(out=xin[:, bi * P:(bi + 1) * P],
                                        in0=out0[:, bi * P:(bi + 1) * P],
                                        scalar1=u0[:, bi:bi + 1], scalar2=None,
                                        op0=OP.mult)
            nc.vector.memset(
                xin[:].rearrange("p (t f) -> p t f", t=nb)[:, :, 0:1], 1.0)

            for li, (ktiles, jtiles) in enumerate(LAYERS):
                Fi, Fo = ktiles * P, jtiles * P
                xinT = wide.tile([P, nb * Fi], BF, tag=f"xinT{li}")
                for bi in range(nb):
                    for kt in range(ktiles):
                        src_ap = xin[:, bi * Fi + kt * P: bi * Fi + (kt + 1) * P]
                        dst_ap = xinT[:, (bi * ktiles + kt) * P:
                                      (bi * ktiles + kt + 1) * P]
                        if c["DMAT"]:
                            nc.sync.dma_start_transpose(out=dst_ap, in_=src_ap)
                        else:
                            tp = psT.tile([P, P], BF, tag="tp")
                            nc.tensor.transpose(out=tp[:], in_=src_ap,
                                                identity=ident_bf[:])
                            nc.any.tensor_copy(out=dst_ap, in_=tp[:])

                t2l = sc.tile([P, CHUNK], F32, tag=f"t2_{li}")
                xnew = wide.tile([P, nb * Fo], BF, tag=f"xin{li + 1}")
                for h0 in range(0, nb, HB):
                    hn = min(HB, nb - h0)
                    ms = []
                    for bi in range(h0, h0 + hn):
                        mps = psM.tile([P, 384], F32, tag="m")
                        for kt in range(ktiles):
                            nc.tensor.matmul(
                                out=mps[:, :Fo],
                                lhsT=xinT[:, (bi * ktiles + kt) * P:
                                          (bi * ktiles + kt + 1) * P],
                                rhs=Wl[li][kt][:],
                                start=(kt == 0), stop=(kt == ktiles - 1))
                        sql = scr.tile([P, 384], BF, tag="sql")
                        nc.scalar.activation(sql[:, :Fo], mps[:, :Fo],
                                             AF.Square,
                                             accum_out=t2l[:, bi:bi + 1])
                        ms.append(mps)
                    g = scale_chain(t2l[:, h0:h0 + hn], hn, f"c{li}")
                    for j, bi in enumerate(range(h0, h0 + hn)):
                        if li >= 1:
                            nc.scalar.activation(
                                xnew[:, bi * Fo:(bi + 1) * Fo], ms[j][:, :Fo],
                                AF.Relu, scale=g[:, j:j + 1])
                        else:
                            nc.vector.tensor_scalar(
                                out=xnew[:, bi * Fo:(bi + 1) * Fo],
                                in0=ms[j][:, :Fo],
                                scalar1=g[:, j:j + 1], scalar2=0.0,
                                op0=OP.mult, op1=OP.max)
                if li < 2:
                    nc.vector.memset(
                        xnew[:].rearrange("p (t f) -> p t f", t=nb)[:, :, 0:1], 1.0)
                xin = xnew

            # pooling partial sums
            for bi in range(nb):
                b = cb0 + bi
                rhs = mask_bf if b == NBLK - 1 else ones_bf
                for jt in range(3):
                    nc.tensor.matmul(
                        out=pool_ps[:, jt:jt + 1],
                        lhsT=xin[:, bi * 384 + jt * P: bi * 384 + (jt + 1) * P],
                        rhs=rhs[:],
                        start=(cb0 == 0 and bi == 0), stop=(b == NBLK - 1),
                        skip_group_check=True)

        # software pipeline: emit chunk c's GIN scatter before chunk c-1's MLP
        # so the PE always has independent ready work queued behind a stall
        prev = None
        for (cb0, nb) in chunks:
            st = emit_gin(cb0, nb)
            if prev is not None:
                emit_mlp(prev[0], prev[1], prev[2], prev[3])
            prev = (cb0, nb, st[0], st[1])
        emit_mlp(prev[0], prev[1], prev[2], prev[3])

        pool_sb = consts.tile([P, 4], F32)
        nc.vector.tensor_copy(out=pool_sb[:, 0:3], in_=pool_ps[:, 0:3])
        nc.sync.dma_start(out=out_d[:], in_=pool_sb[:, 0:3])

    return nc


def _split_excess_waits(nc, mybir, limit=1):
    """Walrus encodes at most one sync-wait on most compute instructions; Tile
    can emit several. Hoist the excess into standalone waits on the same
    engine right before the instruction."""
    keep_types = ("InstEventSemaphore", "InstNoOp", "InstBranch", "InstHalt")
    n = 0
    for fn in nc.m.functions:
        for bb in fn.blocks:
            out = []
            for inst in bb.instructions:
                si = getattr(inst, "sync_info", None)
                tname = type(inst).__name__
                if (si is not None and si.on_wait is not None
                        and len(si.on_wait) > limit and tname not in keep_types):
                    waits = list(si.on_wait)
                    for w in waits[:-limit]:
                        n += 1
                        ev = mybir.InstNoOp(name=f"I-wsplit-{n}")
                        ev.engine = inst.engine
                        ev.sync_info = mybir.SyncInfo(on_wait=[w], on_update=[])
                        out.append(ev)
                    inst.sync_info = mybir.SyncInfo(
                        on_wait=waits[-limit:],
                        on_update=list(si.on_update) if si.on_update else [])
                out.append(inst)
            bb.instructions = out


# ---------------------------------------------------------------------------
# host epilogue (tiny [384] -> outputs, mirrors reference ops in fp32)
# ---------------------------------------------------------------------------

def host_epilogue(total, N, Wc, bc):
    Wc = np.asarray(Wc, np.float32)
    bc = np.asarray(bc, np.float32)
    hm = (total / np.float32(N)).astype(np.float32)
    hm[0] = 0.0
    y0, tail = hm[0:1], hm[1:]
    z = np.maximum(y0 + EPS, 1 + EPS).astype(np.float32)
    dist = np.log(z + np.sqrt(z * z - 1)).astype(np.float32)
    nrm = np.float32(np.sqrt((tail * tail).sum() + EPS))
    xt = np.concatenate([np.zeros(1, np.float32), dist / nrm * tail]).astype(np.float32)
    mx = np.concatenate([xt[:1], xt[1:] @ Wc.T + bc]).astype(np.float32)

    def exp_map(v):
        t2 = (v[1:] ** 2).sum()
        n = np.sqrt(np.clip(t2 + EPS, 1e-6, None))
        ncut = np.minimum(n, 50.0)
        tail_out = np.sinh(ncut) * v[1:] / n
        first = np.sqrt(1 + (tail_out ** 2).sum())
        return np.concatenate([[first], tail_out]).astype(np.float32)

    h_classify = exp_map(mx)
    if np.all(mx == 0):
        h_classify = np.zeros_like(h_classify)
    y0, tailh = h_classify[0:1], h_classify[1:]
    z = np.maximum(y0 + EPS, 1 + EPS).astype(np.float32)
    dist = np.log(z + np.sqrt(z * z - 1)).astype(np.float32)
    nrm = np.float32(np.sqrt((tailh * tailh).sum() + EPS))
    xt2 = np.concatenate([np.zeros(1, np.float32), dist / nrm * tailh]).astype(np.float32)
    e = np.exp(xt2 - xt2.max())
    sm = (e / e.sum()).astype(np.float32)
    sm[0] = 0.0
    prob = exp_map(sm)
    return h_classify, prob


# ---------------------------------------------------------------------------
# entry point
# ---------------------------------------------------------------------------

_CACHE = {}


def kernel(x, edge_index, W0, b0, W1, b1, W2, b2, Wc, bc, _cfg=None, _runner=None,
           _split=True):
    from concourse import mybir
    cfg = dict(DEFAULT_CFG)
    if _cfg:
        cfg.update(_cfg)
    c = _derive(cfg)
    bf_np = mybir.dt.np(mybir.dt.bfloat16)

    xt_bf, Kb, cores = host_prep(x, edge_index, cfg, bf_np)
    wts = prep_weights(W0, b0, W1, b1, W2, b2, bf_np)

    key = (tuple(Kb), tuple(sorted(cfg.items())), _split)
    if key not in _CACHE:
        nc = build_program(Kb, cfg)
        if _split:
            # walrus codegen wait-slot legalization (HW path only; CoreSim's
            # race detector rejects the bare EventSemaphores)
            _split_excess_waits(nc, mybir)
        _CACHE[key] = nc
    nc = _CACHE[key]

    in_maps = []
    for ci in range(c["NCORES"]):
        cd = cores[ci]
        in_maps.append(dict(xt=xt_bf, idx=cd["idx"], slot=cd["slot"],
                            own=cd["own"], **wts))

    if _runner is not None:
        results = _runner(nc, in_maps)
    else:
        from concourse.bass_utils import run_bass_kernel_spmd
        res = run_bass_kernel_spmd(nc, in_maps, core_ids=list(range(c["NCORES"])))
        results = res.results

    total = np.zeros(384, np.float64)
    for ci in range(c["NCORES"]):
        out = np.asarray(results[ci]["out"])   # [128, 3] feat-major
        total += out.T.reshape(384).astype(np.float64)
    total = total.astype(np.float32)

    h_classify, prob = host_epilogue(total, c["N"], Wc, bc)
    return h_classify, prob
